# revision 1
# baseline (speedup 1.0000x reference)
"""Trainium2 Bass kernel for ragged subword mean pooling (nn_Bert).

Problem: out[b, j] = mean(bert_embedding[b, st_j:ed_j]) if (mask & ed>st) else 0
Shapes: bert_embedding [32, 1024, 768] f32, x_bert_offset [32, 768, 2] i32,
        x_mask [32, 768] i32 -> out [32, 768, 768] f32.

Strategy (pure data parallel, 4 batch rows per core on 8 cores):
  Spans are contiguous sorted segments, so per row the pooling is
  out = A.T @ E with A[s, j] = 1 iff st_j <= s < ed_j, followed by a
  per-word scale (valid/len). A is built on-chip from iota-vs-boundary
  compares (DVE), the contraction runs on the PE in float32r (full rate),
  and the scale is folded into the PSUM->SBUF drain on the scalar engine.
  Only the (m, k) tile pairs where words of m-tile intersect positions of
  k-tile are computed; the active-pair hull is derived on the host from
  the actual offsets (a superset is always correct since A is 0 outside).
"""

import sys

if "/opt/trn_rl_repo" not in sys.path:
    sys.path.insert(0, "/opt/trn_rl_repo")

import numpy as np

B, S, W, D = 32, 1024, 768, 768
NCORES = 8
RPC = B // NCORES  # rows per core
KT = S // 128  # 8 k-tiles (positions)
MT = W // 128  # 6 m-tiles (words)

_CACHE = {}


def _active_pairs(st, ed):
    """Per row-slot r, the hull of active (m, k) tile pairs, unioned over
    cores (the SPMD program is shared by all cores).

    Returns pairs[r][m] = list of k-tiles to contract for m-tile m.
    """
    pairs = []
    for r in range(RPC):
        per_m = []
        for m in range(MT):
            klo, khi = KT, 0
            for c in range(NCORES):
                b = c * RPC + r
                s0 = int(st[b, m * 128 : (m + 1) * 128].min())
                s1 = int(ed[b, m * 128 : (m + 1) * 128].max())
                if s1 > s0:
                    klo = min(klo, s0 // 128)
                    khi = max(khi, (s1 + 127) // 128)
            per_m.append(list(range(klo, khi)) if khi > klo else [])
        pairs.append(per_m)
    return pairs


def build_program(pairs, repeat=1):
    """Build the SPMD Bass program (one program, run on all 8 cores)."""
    import concourse.tile as tile
    from concourse import bacc, mybir

    f32 = mybir.dt.float32
    f32r = mybir.dt.float32r
    i32 = mybir.dt.int32
    AF = mybir.ActivationFunctionType
    OP = mybir.AluOpType

    nc = bacc.Bacc(
        "TRN2", target_bir_lowering=False, debug=False, num_devices=NCORES
    )

    E_in = nc.dram_tensor("E_in", [RPC, S, D], f32r, kind="ExternalInput").ap()
    st_in = nc.dram_tensor("st_in", [RPC, W], f32r, kind="ExternalInput").ap()
    ed_in = nc.dram_tensor("ed_in", [RPC, W], f32r, kind="ExternalInput").ap()
    sc_in = nc.dram_tensor("sc_in", [RPC, W], f32, kind="ExternalInput").ap()
    ones_in = nc.dram_tensor("ones_in", [1, 128], f32r, kind="ExternalInput").ap()
    out = nc.dram_tensor("out", [RPC, W, D], f32, kind="ExternalOutput").ap()

    with tile.TileContext(nc) as tc:
        with (
            tc.tile_pool(name="const", bufs=1) as cpool,
            tc.tile_pool(name="E", bufs=2 * KT) as epool,
            tc.tile_pool(name="bc", bufs=2) as bcpool,
            tc.tile_pool(name="A", bufs=6) as apool,
            tc.tile_pool(name="outsb", bufs=4) as opool,
            tc.tile_pool(name="psum", bufs=2, space="PSUM") as pspool,
            tc.tile_pool(name="psbc", bufs=1, space="PSUM") as psbc,
        ):
            # constants
            ones = cpool.tile([1, 128], f32r)
            nc.sync.dma_start(ones[:], ones_in[:])
            io_i = cpool.tile([128, KT], i32)
            nc.gpsimd.iota(io_i[:], pattern=[[128, KT]], base=0, channel_multiplier=1)
            io_f = cpool.tile([128, KT], f32)
            nc.vector.tensor_copy(io_f[:], io_i[:])

            for _ in range(repeat):
                for r in range(RPC):
                    # stream the embedding row in k-tiles
                    et = []
                    for k in range(KT):
                        t = epool.tile([128, D], f32r, tag="E")
                        nc.sync.dma_start(
                            t[:], E_in[r, k * 128 : (k + 1) * 128, :]
                        )
                        et.append(t)

                    # boundary rows -> broadcast along partitions via K=1 matmul
                    st_row = bcpool.tile([1, W], f32r, tag="row")
                    ed_row = bcpool.tile([1, W], f32r, tag="row")
                    nc.sync.dma_start(st_row[:], st_in[r : r + 1, :])
                    nc.sync.dma_start(ed_row[:], ed_in[r : r + 1, :])
                    bc_ps = psbc.tile([128, 2 * W], f32)
                    for i, row in enumerate((st_row, ed_row)):
                        for n0 in range(0, W, 512):
                            n1 = min(n0 + 512, W)
                            nc.tensor.matmul(
                                bc_ps[:, i * W + n0 : i * W + n1],
                                ones[:],
                                row[:, n0:n1],
                                start=True,
                                stop=True,
                            )
                    st_b = bcpool.tile([128, W], f32, tag="bcast")
                    ed_b = bcpool.tile([128, W], f32, tag="bcast")
                    nc.scalar.activation(st_b[:], bc_ps[:, 0:W], AF.Copy)
                    nc.scalar.activation(ed_b[:], bc_ps[:, W : 2 * W], AF.Copy)

                    # per-word scale as per-partition column per m-tile
                    sc_col = bcpool.tile([128, MT], f32, tag="scol")
                    nc.sync.dma_start(
                        sc_col[:], sc_in[r].rearrange("(m p) -> p m", p=128)
                    )

                    for m in range(MT):
                        klist = pairs[r][m]
                        osb = opool.tile([128, D], f32, tag="osb")
                        if not klist:
                            nc.vector.memset(osb[:], 0.0)
                        else:
                            ps = pspool.tile([128, D], f32, tag="ps")
                            for ki, k in enumerate(klist):
                                # A[p, j] = (st_j <= s) * (ed_j > s), s = 128k+p
                                ts1 = apool.tile([128, 128], f32, tag="ts1")
                                nc.vector.tensor_scalar(
                                    ts1[:],
                                    st_b[:, m * 128 : (m + 1) * 128],
                                    io_f[:, k : k + 1],
                                    None,
                                    OP.is_le,
                                )
                                A = apool.tile([128, 128], f32r, tag="A")
                                nc.vector.scalar_tensor_tensor(
                                    A[:],
                                    ed_b[:, m * 128 : (m + 1) * 128],
                                    io_f[:, k : k + 1],
                                    ts1[:],
                                    OP.is_gt,
                                    OP.mult,
                                )
                                first = ki == 0
                                last = ki == len(klist) - 1
                                for n0 in range(0, D, 512):
                                    n1 = min(n0 + 512, D)
                                    nc.tensor.matmul(
                                        ps[:, n0:n1],
                                        A[:],
                                        et[k][:, n0:n1],
                                        start=first,
                                        stop=last,
                                    )
                            nc.scalar.activation(
                                osb[:], ps[:], AF.Copy, scale=sc_col[:, m : m + 1]
                            )
                        nc.sync.dma_start(out[r, m * 128 : (m + 1) * 128, :], osb[:])

    nc.compile()
    return nc


def _prep(bert_embedding, x_bert_offset, x_mask):
    st = x_bert_offset[..., 0].astype(np.int64)
    ed = x_bert_offset[..., 1].astype(np.int64)
    length = ed - st
    valid = (x_mask > 0) & (length > 0)
    scale = np.where(
        valid, 1.0 / np.maximum(length, 1).astype(np.float64), 0.0
    ).astype(np.float32)
    st_f = st.astype(np.float32)
    ed_f = ed.astype(np.float32)
    pairs = _active_pairs(st, ed)
    in_maps = []
    ones = np.ones((1, 128), dtype=np.float32)
    E = np.ascontiguousarray(bert_embedding, dtype=np.float32)
    for c in range(NCORES):
        rows = slice(c * RPC, (c + 1) * RPC)
        in_maps.append(
            {
                "E_in": E[rows],
                "st_in": st_f[rows],
                "ed_in": ed_f[rows],
                "sc_in": scale[rows],
                "ones_in": ones,
            }
        )
    return pairs, in_maps


def kernel(bert_embedding, x_bert_offset, x_mask):
    from concourse.bass_utils import run_bass_kernel_spmd

    pairs, in_maps = _prep(bert_embedding, x_bert_offset, x_mask)
    key = tuple(tuple(tuple(km) for km in pr) for pr in pairs)
    nc = _CACHE.get(key)
    if nc is None:
        nc = build_program(pairs)
        _CACHE[key] = nc
    res = run_bass_kernel_spmd(nc, in_maps, list(range(NCORES)))
    out = np.concatenate([res.results[c]["out"] for c in range(NCORES)], axis=0)
    return out.astype(np.float32)


# revision 17
# speedup vs baseline: 19.8620x; 19.8620x over previous
"""Trainium2 Bass kernel for ragged subword mean pooling (nn_Bert).

Problem: out[b, j] = mean(bert_embedding[b, st_j:ed_j]) if (mask & ed>st) else 0
Shapes: bert_embedding [32, 1024, 768] f32, x_bert_offset [32, 768, 2] i32,
        x_mask [32, 768] i32 -> out [32, 768, 768] f32.

Strategy (pure data parallel, 4 batch rows per core on 8 cores):
  Spans are contiguous sorted segments, so per row the pooling is
  out = A.T @ E where A[s, j] = scale_j iff st_j <= s < ed_j
  (scale_j = valid/len folds the mean and mask directly into A).
  Each position s belongs to at most ONE word, so every A tile has at
  most one nonzero per partition row. The host ships just that
  (column, value) pair per position (~32KB/core) and the device
  reconstructs each [128, win] A window in a single fused DVE op
  against a constant column-index tile J:
      A[p, j] = (J[p, j] == idx_p) * val_p
  The contraction runs on the PE in float32r (full rate; values are
  rounded to ~tf32, rel err ~1e-4). PSUM is drained by plain scalar-
  engine copies. Only (m, k) tile pairs whose word/position ranges
  intersect are computed; the active-pair hull is derived on the host
  from the actual offsets (a superset is always correct since A is 0
  outside).
"""

import sys

if "/opt/trn_rl_repo" not in sys.path:
    sys.path.insert(0, "/opt/trn_rl_repo")

import numpy as np

B, S, W, D = 32, 1024, 768, 768
NCORES = 8
RPC = B // NCORES  # rows per core
KT = S // 128  # 8 k-tiles (positions)
MT = W // 128  # 6 m-tiles (words)

_CACHE = {}


def _active_pairs(st, ed):
    """Per row-slot r: hull of active k-tiles for each m-tile, and hull of
    active m-tiles for each k-tile, unioned over cores (the SPMD program is
    shared by all 8 cores). A superset only costs time, never correctness.
    """
    kl = []
    for r in range(RPC):
        per_m = []
        for m in range(MT):
            klo, khi = KT, 0
            for c in range(NCORES):
                b = c * RPC + r
                s0 = int(st[b, m * 128 : (m + 1) * 128].min())
                s1 = int(ed[b, m * 128 : (m + 1) * 128].max())
                if s1 > s0:
                    klo = min(klo, s0 // 128)
                    khi = max(khi, (s1 + 127) // 128)
            per_m.append((klo, khi) if khi > klo else None)
        kl.append(per_m)

    mw = []
    for r in range(RPC):
        per_k = []
        for k in range(KT):
            mlo, mhi = MT, 0
            for m in range(MT):
                if kl[r][m] and kl[r][m][0] <= k < kl[r][m][1]:
                    mlo = min(mlo, m)
                    mhi = max(mhi, m + 1)
            per_k.append((mlo, mhi) if mhi > mlo else None)
        mw.append(per_k)
    return kl, mw


def build_program(pairs, repeat=1, drain="act", io="ext", stage=3, nodma=False,
                  ebufs=5, abufs=6, psbufs=3, obufs=4, avbufs=2):
    """Build the SPMD Bass program (one program, run on all 8 cores)."""
    import concourse.tile as tile
    from concourse import bacc, mybir

    kl, mw = pairs
    f32 = mybir.dt.float32
    f32r = mybir.dt.float32r
    i32 = mybir.dt.int32
    AF = mybir.ActivationFunctionType
    OP = mybir.AluOpType

    nc = bacc.Bacc(
        "TRN2", target_bir_lowering=False, debug=False, num_devices=NCORES
    )

    E_in = nc.dram_tensor("E_in", [RPC, S, D], f32r, kind="ExternalInput").ap()
    # packed per (r, k): column 2*(r*KT+k) = one-hot column index within the
    # A window (or -1), column +1 = A value (scale of the word at that
    # position, 0 if masked/empty/uncovered)
    av_in = nc.dram_tensor("av_in", [128, RPC * KT * 2], f32, kind="ExternalInput").ap()
    if io == "ext":
        out = nc.dram_tensor("out", [RPC, W, D], f32, kind="ExternalOutput").ap()
        tok = None
    else:
        out = nc.dram_tensor("out_scratch", [RPC, W, D], f32).ap()
        tok = nc.dram_tensor("tok", [128, 16], f32, kind="ExternalOutput").ap()
    outdma = not nodma

    def win(r, k):
        if mw[r][k] is None:
            return None
        mlo, mhi = mw[r][k]
        return mlo * 128, (mhi - mlo) * 128

    awidth = 128
    for r in range(RPC):
        for k in range(KT):
            if mw[r][k]:
                awidth = max(awidth, (mw[r][k][1] - mw[r][k][0]) * 128)

    any_empty_m = any(kl[r][m] is None for r in range(RPC) for m in range(MT))

    with tile.TileContext(nc) as tc:
        with (
            tc.tile_pool(name="const", bufs=1) as cpool,
            tc.tile_pool(name="E", bufs=ebufs) as epool,
            tc.tile_pool(name="bc", bufs=avbufs) as bcpool,
            tc.tile_pool(name="A", bufs=abufs) as apool,
            tc.tile_pool(name="outsb", bufs=obufs) as opool,
            tc.tile_pool(name="psum", bufs=psbufs, space="PSUM") as pspool,
        ):
            # constant column-index tile J[p, j] = j
            j_i = cpool.tile([128, awidth], i32)
            nc.gpsimd.iota(j_i[:], pattern=[[1, awidth]], base=0, channel_multiplier=0)
            j_f = cpool.tile([128, awidth], f32)
            nc.vector.tensor_copy(j_f[:], j_i[:])
            if any_empty_m or stage < 3:
                zeros = cpool.tile([128, D], f32)
                nc.vector.memset(zeros[:], 0.0)
            econst = None
            if nodma:
                econst = []
                for h in range(2):
                    tt = cpool.tile([128, 4 * D], f32r, tag=f"Ec{h}")
                    nc.vector.memset(tt[:].bitcast(f32), 0.5)
                    econst.append(tt)

            last_at = None
            for _ in range(repeat):
                if stage >= 0:
                    av = bcpool.tile([128, RPC * KT * 2], f32, tag="av")
                    nc.sync.dma_start(av[:], av_in[:, :])

                for r in range(RPC):
                    # E row in two batched DMAs of 4 k-tiles each
                    et = []
                    if nodma:
                        for k4 in range(KT):
                            et.append(econst[k4 // 4][:, (k4 % 4) * D : (k4 % 4 + 1) * D])
                    else:
                        for h in range(2):
                            t = epool.tile([128, 4 * D], f32r, tag="E")
                            src = E_in[r, h * 512 : (h + 1) * 512, :].rearrange(
                                "(k p) d -> p k d", p=128
                            )
                            nc.sync.dma_start(
                                t[:].rearrange("p (k d) -> p k d", d=D), src
                            )
                            for k4 in range(4):
                                et.append(t[:, k4 * D : (k4 + 1) * D])

                    # one-hot A windows, one fused DVE op per k-tile
                    ak = {}
                    for k in range(KT if stage >= 1 else 0):
                        w = win(r, k)
                        if w is None:
                            continue
                        j0, wd = w
                        c = (r * KT + k) * 2
                        at = apool.tile([128, awidth], f32r, tag="A")
                        nc.vector.tensor_scalar(
                            at[:, :wd],
                            j_f[:, :wd],
                            av[:, c : c + 1],
                            av[:, c + 1 : c + 2],
                            OP.is_equal,
                            OP.mult,
                        )
                        ak[k] = (at, j0)
                        last_at = at

                    for m in range(MT):
                        if kl[r][m] is None or stage < 2:
                            if outdma:
                                nc.sync.dma_start(
                                    out[r, m * 128 : (m + 1) * 128, :], zeros[:]
                                )
                            continue
                        klo, khi = kl[r][m]
                        ps = pspool.tile([128, D], f32, tag="ps")
                        for k in range(klo, khi):
                            at, j0 = ak[k]
                            lhsT = at[:, m * 128 - j0 : (m + 1) * 128 - j0]
                            first = k == klo
                            last = k == khi - 1
                            for n0 in range(0, D, 512):
                                n1 = min(n0 + 512, D)
                                nc.tensor.matmul(
                                    ps[:, n0:n1],
                                    lhsT,
                                    et[k][:, n0:n1],
                                    start=first,
                                    stop=last,
                                )
                        if stage < 3:
                            if outdma:
                                nc.sync.dma_start(
                                    out[r, m * 128 : (m + 1) * 128, :], zeros[:]
                                )
                            continue
                        osb = opool.tile([128, D], f32, tag="osb")
                        if drain == "act":
                            nc.scalar.activation(osb[:], ps[:], AF.Copy)
                        else:
                            nc.vector.tensor_copy(osb[:], ps[:])
                        if outdma:
                            nc.sync.dma_start(
                                out[r, m * 128 : (m + 1) * 128, :], osb[:]
                            )

            if tok is not None:
                if last_at is not None:
                    nc.sync.dma_start(tok[:], last_at[:, :16].bitcast(f32))
                else:
                    nc.sync.dma_start(tok[:], zeros[:, :16])

    nc.compile()
    return nc


def _prep(bert_embedding, x_bert_offset, x_mask):
    st = x_bert_offset[..., 0].astype(np.int64)
    ed = x_bert_offset[..., 1].astype(np.int64)
    length = ed - st
    valid = (x_mask > 0) & (length > 0)
    scale = np.where(
        valid, 1.0 / np.maximum(length, 1).astype(np.float64), 0.0
    ).astype(np.float32)
    st_ext = np.concatenate([st, ed[:, -1:]], axis=1)  # [B, W+1]

    # word index of each position (-1 if uncovered)
    word_of = np.full((B, S), -1, dtype=np.int64)
    s_idx = np.arange(S)
    for b in range(B):
        j = np.searchsorted(st_ext[b], s_idx, side="right") - 1
        ok = (j >= 0) & (j < W)
        word_of[b] = np.where(ok, j, -1)

    pairs = _active_pairs(st, ed)
    kl, mw = pairs

    E = np.ascontiguousarray(bert_embedding, dtype=np.float32)
    in_maps = []
    for c in range(NCORES):
        av = np.zeros((128, RPC * KT * 2), dtype=np.float32)
        for r in range(RPC):
            b = c * RPC + r
            for k in range(KT):
                if mw[r][k] is None:
                    continue
                j0 = mw[r][k][0] * 128
                col = (r * KT + k) * 2
                s = k * 128 + np.arange(128)
                wj = word_of[b, s]
                covered = wj >= 0
                # window hull guarantees covered words lie inside [j0, j0+wd)
                av[:, col] = np.where(covered, wj - j0, -1).astype(np.float32)
                av[:, col + 1] = np.where(
                    covered, scale[b, np.clip(wj, 0, W - 1)], 0.0
                )
        in_maps.append(
            {
                "E_in": E[c * RPC : (c + 1) * RPC],
                "av_in": av,
            }
        )
    return pairs, in_maps


def kernel(bert_embedding, x_bert_offset, x_mask):
    from concourse.bass_utils import run_bass_kernel_spmd

    bert_embedding = np.asarray(bert_embedding, dtype=np.float32)
    x_bert_offset = np.asarray(x_bert_offset)
    x_mask = np.asarray(x_mask)
    pairs, in_maps = _prep(bert_embedding, x_bert_offset, x_mask)
    key = repr(pairs)
    nc = _CACHE.get(key)
    if nc is None:
        nc = build_program(pairs)
        _CACHE[key] = nc
    res = run_bass_kernel_spmd(nc, in_maps, list(range(NCORES)))
    out = np.concatenate([res.results[c]["out"] for c in range(NCORES)], axis=0)
    return out.astype(np.float32)
